# revision 10
# baseline (speedup 1.0000x reference)
"""CapsuleLayer dynamic-routing kernel for Trainium2 (8 NeuronCores).

Strategy: data-parallel over batch B (64 -> 8 per core), zero communication.
Per core:
  u_hat[b,j,n,d] = sum_i x[b,n,i] W[j,n,d,i] computed on the PE as 128
  full-rank matmuls: stationary = host-built block-diagonal x
  [(n16,i8)=128 x (n16,b8)=128], moving = W slice [(n16,i8)=128 x (j,d)=512],
  so each matmul yields u_hat for 16 n's, all 8 local b's, all (j,d).
  Routing (3 iterations) runs on DVE/GpSimd/ACT with a PE ones-reduction
  over n for the softmax-weighted sum s.
"""

import numpy as np
import ml_dtypes

from concourse import bass
import concourse.mybir as mybir
import concourse.bacc as bacc
import concourse.tile as tile
from concourse.bass_utils import run_bass_kernel_spmd

BF16 = mybir.dt.bfloat16
F32 = mybir.dt.float32
AF = mybir.ActivationFunctionType
ALU = mybir.AluOpType
AX = mybir.AxisListType

B, N, I, J, D = 64, 2048, 8, 32, 16
NCORES = 8
BL = B // NCORES          # 8 local batches
KC = N // 16              # 128 contraction chunks of 16 n's
JD = J * D                # 512
GRP = 4                   # kc's per DMA batch
NG = KC // GRP            # 32
BLK = 4                   # kc's per routing block
NBLK = KC // BLK          # 32
EPS = 1e-7


def _build_nc(reps=1):
    nc = bacc.Bacc("TRN2", target_bir_lowering=False)
    xbd_d = nc.declare_dram_parameter("xbd", [NG, 128, GRP, 128], BF16, False)
    wm_d = nc.declare_dram_parameter("wm", [NG, 128, GRP, JD], BF16, False)
    ones_d = nc.declare_dram_parameter("onesbd", [128, BL], BF16, False)
    vout_d = nc.declare_dram_parameter("vout", [BL, JD], F32, True)

    for _ in range(reps):
        _emit_body(nc, xbd_d, wm_d, ones_d, vout_d)
    nc.compile()
    return nc


def _emit_body(nc, xbd_d, wm_d, ones_d, vout_d):
    with tile.TileContext(nc) as tc:
        with (
            tc.tile_pool(name="big", bufs=1) as big,
            tc.tile_pool(name="wpool", bufs=3) as wpool,
            tc.tile_pool(name="xpool", bufs=3) as xpool,
            tc.tile_pool(name="ppool", bufs=4, space="PSUM") as ppool,
            tc.tile_pool(name="spool", bufs=1, space="PSUM") as spool,
            tc.tile_pool(name="ypool", bufs=2) as ypool,
            tc.tile_pool(name="small", bufs=1) as small,
        ):
            # persistent SBUF
            U = big.tile([128, D, KC, J], BF16, tag="U")        # [(n16,b8), d, kc, j]
            blg = big.tile([128, KC, J], BF16, tag="blg")       # routing logits
            expb = big.tile([128, KC, J], BF16, tag="expb")
            vrep = big.tile([128, D, J], BF16, tag="vrep")
            onesbd = small.tile([128, BL], BF16, tag="ones")
            nc.sync.dma_start(out=onesbd[:], in_=ones_d[:])

            # ---- Phase 1: u_hat ----
            for g in range(NG):
                wt = wpool.tile([128, GRP, JD], BF16, tag="w")
                xt = xpool.tile([128, GRP, 128], BF16, tag="x")
                nc.sync.dma_start(out=wt[:], in_=wm_d[g])
                nc.sync.dma_start(out=xt[:], in_=xbd_d[g])
                for q in range(GRP):
                    kc = g * GRP + q
                    pt = ppool.tile([128, JD], F32, tag="p1")
                    nc.tensor.matmul(
                        pt[:], lhsT=xt[:, q, :], rhs=wt[:, q, :],
                        start=True, stop=True,
                    )
                    src = pt[:].rearrange("p (j d) -> p d j", j=J, d=D)
                    dst = U[:, :, kc, :]
                    if kc % 4 == 0:
                        nc.vector.tensor_copy(dst, src)
                    else:
                        nc.scalar.copy(dst, src)

            # ---- Phase 2: routing ----
            for it in range(3):
                if it > 0:
                    # c = softmax(blg) over j; fold 1/Z into expb (in place)
                    nc.scalar.activation(expb[:], blg[:], AF.Exp)
                    zs = small.tile([128, KC], F32, tag="zs")
                    nc.vector.tensor_reduce(zs[:], expb[:], axis=AX.X, op=ALU.add)
                    zr = small.tile([128, KC], F32, tag="zr")
                    nc.vector.reciprocal(zr[:], zs[:])
                    zin = zr[:].unsqueeze(2).broadcast_to([128, KC, J])
                    nc.vector.tensor_mul(expb[:], expb[:], zin)

                # s[b,(d,j)] = sum_n c * u_hat  (PE ones-reduction over n)
                spt = spool.tile([128, JD], F32, tag="s")
                for blk in range(NBLK):
                    eng = nc.gpsimd if (blk % 4) == 3 else nc.vector
                    sl = slice(blk * BLK, (blk + 1) * BLK)
                    if it > 0:
                        yt = ypool.tile([128, D, BLK, J], BF16, tag="y")
                        e_in = (
                            expb[:, sl, :]
                            .unsqueeze(1)
                            .broadcast_to([128, D, BLK, J])
                        )
                        eng.tensor_mul(yt[:], U[:, :, sl, :], e_in)
                    for q in range(BLK):
                        kc = blk * BLK + q
                        rhs = U[:, :, kc, :] if it == 0 else yt[:, :, q, :]
                        nc.tensor.matmul(
                            spt[0:BL, :],
                            lhsT=onesbd[:],
                            rhs=rhs,
                            start=(kc == 0),
                            stop=(kc == KC - 1),
                        )

                # squash: v = s / sqrt(sum_d s^2 + eps)
                s_sb = small.tile([BL, J, D], F32, tag="ssb")
                src = spt[0:BL, :].rearrange("p (d j) -> p j d", j=J, d=D)
                nc.scalar.mul(s_sb[:], src, (1.0 / J) if it == 0 else 1.0)
                sq = small.tile([BL, J, D], F32, tag="sq")
                nc.vector.tensor_mul(sq[:], s_sb[:], s_sb[:])
                ssq = small.tile([BL, J], F32, tag="ssq")
                nc.vector.tensor_reduce(ssq[:], sq[:], axis=AX.X, op=ALU.add)
                ssqe = small.tile([BL, J], F32, tag="ssqe")
                nc.vector.tensor_scalar_add(ssqe[:], ssq[:], EPS)
                sr = small.tile([BL, J], F32, tag="sr")
                nc.scalar.sqrt(sr[:], ssqe[:])
                rden = small.tile([BL, J], F32, tag="rden")
                nc.vector.reciprocal(rden[:], sr[:])

                if it == 2:
                    vf = small.tile([BL, J, D], F32, tag="vf")
                    den_in = rden[:].unsqueeze(2).broadcast_to([BL, J, D])
                    nc.vector.tensor_mul(vf[:], s_sb[:], den_in)
                    nc.sync.dma_start(
                        out=vout_d[:], in_=vf[:].rearrange("p j d -> p (j d)")
                    )
                else:
                    # v in (d, j) order, bf16, for broadcast against U
                    vb = small.tile([BL, D, J], BF16, tag="vb")
                    den_in2 = rden[:].unsqueeze(1).broadcast_to([BL, D, J])
                    nc.vector.tensor_mul(
                        vb[:], s_sb[:].transpose([0, 2, 1]), den_in2
                    )
                    # replicate v across the 16 n-sub partitions
                    for k in range(16):
                        nc.sync.dma_start(
                            out=vrep[k * BL:(k + 1) * BL, :, :], in_=vb[:]
                        )
                    # blg += sum_d u_hat * v
                    for blk in range(NBLK):
                        eng = nc.gpsimd if (blk % 4) == 3 else nc.vector
                        sl = slice(blk * BLK, (blk + 1) * BLK)
                        p2 = ypool.tile([128, D, BLK, J], BF16, tag="p2")
                        vin = (
                            vrep[:]
                            .unsqueeze(2)
                            .broadcast_to([128, D, BLK, J])
                        )
                        eng.tensor_mul(p2[:], U[:, :, sl, :], vin)
                        t8 = ypool.tile([128, 8, BLK, J], BF16, tag="t8")
                        eng.tensor_add(t8[:], p2[:, 0:8], p2[:, 8:16])
                        t4 = ypool.tile([128, 4, BLK, J], BF16, tag="tr")
                        eng.tensor_add(t4[:], t8[:, 0:4], t8[:, 4:8])
                        t2 = ypool.tile([128, 2, BLK, J], BF16, tag="tr2")
                        eng.tensor_add(t2[:], t4[:, 0:2], t4[:, 2:4])
                        t1 = ypool.tile([128, 1, BLK, J], BF16, tag="tr1")
                        eng.tensor_add(t1[:], t2[:, 0:1], t2[:, 1:2])
                        if it == 0:
                            eng.tensor_copy(blg[:, sl, :], t1[:].squeeze(1))
                        else:
                            eng.tensor_add(
                                blg[:, sl, :], blg[:, sl, :], t1[:].squeeze(1)
                            )


_NC_CACHE = None


def _get_nc():
    global _NC_CACHE
    if _NC_CACHE is None:
        _NC_CACHE = _build_nc()
    return _NC_CACHE


def _prep_inputs(x, W):
    bf = ml_dtypes.bfloat16
    # wm[kc, (n16,i8), (j,d)] = W[j, kc*16+n16, d, i], grouped by GRP for DMA
    Wr = np.asarray(W, np.float32).reshape(J, KC, 16, D, I)
    wm = Wr.transpose(1, 2, 4, 0, 3).reshape(KC, 128, JD)
    wm = np.ascontiguousarray(
        wm.reshape(NG, GRP, 128, JD).transpose(0, 2, 1, 3).astype(bf)
    )
    # ones_bd[(n16,b8), b'] = delta_{b,b'}
    onesbd = np.ascontiguousarray(
        np.tile(np.eye(BL, dtype=np.float32), (16, 1)).astype(bf)
    )
    in_maps = []
    xr = np.asarray(x, np.float32).reshape(NCORES, BL, KC, 16, I)
    for c in range(NCORES):
        xbd = np.zeros((KC, 16, I, 16, BL), np.float32)
        idx = np.arange(16)
        # xbd[kc, n, i, n, b] = x[c, b, kc, n, i]
        xbd[:, idx, :, idx, :] = xr[c].transpose(2, 1, 3, 0)
        xbd = xbd.reshape(KC, 128, 128)
        xbd = np.ascontiguousarray(
            xbd.reshape(NG, GRP, 128, 128).transpose(0, 2, 1, 3).astype(bf)
        )
        in_maps.append({"xbd": xbd, "wm": wm, "onesbd": onesbd})
    return in_maps


def kernel(x, W):
    nc = _get_nc()
    in_maps = _prep_inputs(x, W)
    res = run_bass_kernel_spmd(nc, in_maps, list(range(NCORES)))
    outs = [res.results[c]["vout"].reshape(BL, J, D) for c in range(NCORES)]
    return np.concatenate(outs, axis=0).astype(np.float32)


# revision 11
# speedup vs baseline: 1.6679x; 1.6679x over previous
"""CapsuleLayer dynamic-routing kernel for Trainium2 (8 NeuronCores).

Strategy: data-parallel over batch B (64 -> 8 per core), zero communication.
Per core:
  u_hat[b,j,n,d] = sum_i x[b,n,i] W[j,n,d,i] computed on the PE as 128
  full-rank matmuls: stationary = host-built block-diagonal x
  [(n16,i8)=128 x (n16,b8)=128], moving = W slice [(n16,i8)=128 x (j,d)=512],
  so each matmul yields u_hat for 16 n's, all 8 local b's, all (j,d).
  Routing (3 iterations) runs on DVE/GpSimd/ACT with a PE ones-reduction
  over n for the softmax-weighted sum s.
"""

import numpy as np
import ml_dtypes

from concourse import bass
import concourse.mybir as mybir
import concourse.bacc as bacc
import concourse.tile as tile
from concourse.bass_utils import run_bass_kernel_spmd

BF16 = mybir.dt.bfloat16
F32 = mybir.dt.float32
AF = mybir.ActivationFunctionType
ALU = mybir.AluOpType
AX = mybir.AxisListType

B, N, I, J, D = 64, 2048, 8, 32, 16
NCORES = 8
BL = B // NCORES          # 8 local batches
KC = N // 16              # 128 contraction chunks of 16 n's
JD = J * D                # 512
GRP = 4                   # kc's per DMA batch
NG = KC // GRP            # 32
BLK = 4                   # kc's per routing block
NBLK = KC // BLK          # 32
EPS = 1e-7


def _build_nc(reps=1):
    nc = bacc.Bacc("TRN2", target_bir_lowering=False)
    xbd_d = nc.declare_dram_parameter("xbd", [NG, 128, GRP, 128], BF16, False)
    wm_d = nc.declare_dram_parameter("wm", [NG, 128, GRP, JD], BF16, False)
    ones_d = nc.declare_dram_parameter("onesbd", [128, BL], BF16, False)
    vout_d = nc.declare_dram_parameter("vout", [BL, JD], F32, True)

    for _ in range(reps):
        _emit_body(nc, xbd_d, wm_d, ones_d, vout_d)
    nc.compile()
    return nc


def _emit_body(nc, xbd_d, wm_d, ones_d, vout_d):
    with tile.TileContext(nc) as tc:
        with (
            tc.tile_pool(name="big", bufs=1) as big,
            tc.tile_pool(name="wpool", bufs=4) as wpool,
            tc.tile_pool(name="xpool", bufs=3) as xpool,
            tc.tile_pool(name="ppool", bufs=6, space="PSUM") as ppool,
            tc.tile_pool(name="spool", bufs=1, space="PSUM") as spool,
            tc.tile_pool(name="ypool", bufs=2) as ypool,
            tc.tile_pool(name="small", bufs=1) as small,
        ):
            # persistent SBUF
            U = big.tile([128, D, KC, J], BF16, tag="U")        # [(n16,b8), d, kc, j]
            blg = big.tile([128, KC, J], BF16, tag="blg")       # routing logits
            expb = big.tile([128, KC, J], BF16, tag="expb")
            vrep = big.tile([128, D, J], BF16, tag="vrep")
            onesbd = small.tile([128, BL], BF16, tag="ones")
            nc.sync.dma_start(out=onesbd[:], in_=ones_d[:])

            # ---- Phase 1: u_hat ----
            for g in range(NG):
                wt = wpool.tile([128, GRP, JD], BF16, tag="w")
                xt = xpool.tile([128, GRP, 128], BF16, tag="x")
                nc.sync.dma_start(out=wt[:], in_=wm_d[g])
                nc.sync.dma_start(out=xt[:], in_=xbd_d[g])
                for q in range(GRP):
                    kc = g * GRP + q
                    pt = ppool.tile([128, JD], F32, tag="p1")
                    nc.tensor.matmul(
                        pt[:], lhsT=xt[:, q, :], rhs=wt[:, q, :],
                        start=True, stop=True,
                    )
                    src = pt[:].rearrange("p (j d) -> p d j", j=J, d=D)
                    dst = U[:, :, kc, :]
                    if kc % 4 == 0:
                        nc.vector.tensor_copy(dst, src)
                    else:
                        nc.scalar.copy(dst, src)

            # ---- Phase 2: routing ----
            for it in range(3):
                if it > 0:
                    # c = softmax(blg) over j; fold 1/Z into expb (in place)
                    nc.scalar.activation(expb[:], blg[:], AF.Exp)
                    zs = small.tile([128, KC], F32, tag="zs")
                    nc.vector.tensor_reduce(zs[:], expb[:], axis=AX.X, op=ALU.add)
                    zr = small.tile([128, KC], F32, tag="zr")
                    nc.vector.reciprocal(zr[:], zs[:])
                    zin = zr[:].unsqueeze(2).broadcast_to([128, KC, J])
                    nc.vector.tensor_mul(expb[:], expb[:], zin)

                # s[b,(d,j)] = sum_n c * u_hat  (PE ones-reduction over n)
                spt = spool.tile([128, JD], F32, tag="s")
                for blk in range(NBLK):
                    eng = nc.gpsimd if (blk % 4) == 3 else nc.vector
                    sl = slice(blk * BLK, (blk + 1) * BLK)
                    if it > 0:
                        yt = ypool.tile([128, D, BLK, J], BF16, tag="y")
                        e_in = (
                            expb[:, sl, :]
                            .unsqueeze(1)
                            .broadcast_to([128, D, BLK, J])
                        )
                        eng.tensor_mul(yt[:], U[:, :, sl, :], e_in)
                    for q in range(BLK):
                        kc = blk * BLK + q
                        rhs = U[:, :, kc, :] if it == 0 else yt[:, :, q, :]
                        nc.tensor.matmul(
                            spt[0:BL, :],
                            lhsT=onesbd[:],
                            rhs=rhs,
                            start=(kc == 0),
                            stop=(kc == KC - 1),
                        )

                # squash: v = s / sqrt(sum_d s^2 + eps)
                s_sb = small.tile([BL, J, D], F32, tag="ssb")
                src = spt[0:BL, :].rearrange("p (d j) -> p j d", j=J, d=D)
                nc.scalar.mul(s_sb[:], src, (1.0 / J) if it == 0 else 1.0)
                sq = small.tile([BL, J, D], F32, tag="sq")
                nc.vector.tensor_mul(sq[:], s_sb[:], s_sb[:])
                ssq = small.tile([BL, J], F32, tag="ssq")
                nc.vector.tensor_reduce(ssq[:], sq[:], axis=AX.X, op=ALU.add)
                ssqe = small.tile([BL, J], F32, tag="ssqe")
                nc.vector.tensor_scalar_add(ssqe[:], ssq[:], EPS)
                sr = small.tile([BL, J], F32, tag="sr")
                nc.scalar.sqrt(sr[:], ssqe[:])
                rden = small.tile([BL, J], F32, tag="rden")
                nc.vector.reciprocal(rden[:], sr[:])

                if it == 2:
                    vf = small.tile([BL, J, D], F32, tag="vf")
                    den_in = rden[:].unsqueeze(2).broadcast_to([BL, J, D])
                    nc.vector.tensor_mul(vf[:], s_sb[:], den_in)
                    nc.sync.dma_start(
                        out=vout_d[:], in_=vf[:].rearrange("p j d -> p (j d)")
                    )
                else:
                    # v in (d, j) order, bf16, for broadcast against U
                    vb = small.tile([BL, D, J], BF16, tag="vb")
                    den_in2 = rden[:].unsqueeze(1).broadcast_to([BL, D, J])
                    nc.vector.tensor_mul(
                        vb[:], s_sb[:].transpose([0, 2, 1]), den_in2
                    )
                    # replicate v across the 16 n-sub partitions
                    for k in range(16):
                        nc.sync.dma_start(
                            out=vrep[k * BL:(k + 1) * BL, :, :], in_=vb[:]
                        )
                    # blg += sum_d u_hat * v
                    for blk in range(NBLK):
                        eng = nc.gpsimd if (blk % 4) == 3 else nc.vector
                        sl = slice(blk * BLK, (blk + 1) * BLK)
                        p2 = ypool.tile([128, D, BLK, J], BF16, tag="p2")
                        vin = (
                            vrep[:]
                            .unsqueeze(2)
                            .broadcast_to([128, D, BLK, J])
                        )
                        eng.tensor_mul(p2[:], U[:, :, sl, :], vin)
                        t8 = ypool.tile([128, 8, BLK, J], BF16, tag="t8")
                        eng.tensor_add(t8[:], p2[:, 0:8], p2[:, 8:16])
                        t4 = ypool.tile([128, 4, BLK, J], BF16, tag="tr")
                        eng.tensor_add(t4[:], t8[:, 0:4], t8[:, 4:8])
                        t2 = ypool.tile([128, 2, BLK, J], BF16, tag="tr2")
                        eng.tensor_add(t2[:], t4[:, 0:2], t4[:, 2:4])
                        t1 = ypool.tile([128, 1, BLK, J], BF16, tag="tr1")
                        eng.tensor_add(t1[:], t2[:, 0:1], t2[:, 1:2])
                        if it == 0:
                            eng.tensor_copy(blg[:, sl, :], t1[:].squeeze(1))
                        else:
                            eng.tensor_add(
                                blg[:, sl, :], blg[:, sl, :], t1[:].squeeze(1)
                            )


_NC_CACHE = None


def _get_nc():
    global _NC_CACHE
    if _NC_CACHE is None:
        _NC_CACHE = _build_nc()
    return _NC_CACHE


def _prep_inputs(x, W):
    bf = ml_dtypes.bfloat16
    # wm[kc, (n16,i8), (j,d)] = W[j, kc*16+n16, d, i], grouped by GRP for DMA
    Wr = np.asarray(W, np.float32).reshape(J, KC, 16, D, I)
    wm = Wr.transpose(1, 2, 4, 0, 3).reshape(KC, 128, JD)
    wm = np.ascontiguousarray(
        wm.reshape(NG, GRP, 128, JD).transpose(0, 2, 1, 3).astype(bf)
    )
    # ones_bd[(n16,b8), b'] = delta_{b,b'}
    onesbd = np.ascontiguousarray(
        np.tile(np.eye(BL, dtype=np.float32), (16, 1)).astype(bf)
    )
    in_maps = []
    xr = np.asarray(x, np.float32).reshape(NCORES, BL, KC, 16, I)
    for c in range(NCORES):
        xbd = np.zeros((KC, 16, I, 16, BL), np.float32)
        idx = np.arange(16)
        # xbd[kc, n, i, n, b] = x[c, b, kc, n, i]
        xbd[:, idx, :, idx, :] = xr[c].transpose(2, 1, 3, 0)
        xbd = xbd.reshape(KC, 128, 128)
        xbd = np.ascontiguousarray(
            xbd.reshape(NG, GRP, 128, 128).transpose(0, 2, 1, 3).astype(bf)
        )
        in_maps.append({"xbd": xbd, "wm": wm, "onesbd": onesbd})
    return in_maps


def kernel(x, W):
    nc = _get_nc()
    in_maps = _prep_inputs(x, W)
    res = run_bass_kernel_spmd(nc, in_maps, list(range(NCORES)))
    outs = [res.results[c]["vout"].reshape(BL, J, D) for c in range(NCORES)]
    return np.concatenate(outs, axis=0).astype(np.float32)
